# revision 8
# baseline (speedup 1.0000x reference)
"""Trainium2 Bass kernel for PixelPropagationModule (per-pixel self-attention).

Math per batch sample b (B=8, C=256, CI=64, N=H*W=3136):
    Q = Wq @ x + bq            [CI, N]
    K = Wk @ x + bk            [CI, N]
    V = Wv @ x + bv            [C,  N]
    score[i, j] = sum_o Q[o, i] K[o, j]          (N x N)
    att = softmax(score, axis=j)
    out = gamma * (V @ att^T) + x                -> [C, N]

Sharding: pure data parallel, one sample per NeuronCore (B == 8 == n_cores).

Device dataflow (per core):
  - Everything is computed in the "transposed score" orientation S^T[j, i] so
    that the attention weights come out of the PE array with j (the
    contraction index of the second matmul) on partitions; no on-chip
    transposes are needed anywhere.
  - softmax without max subtraction (|score| <= ~40 here, exp is safe in
    fp32/bf16 range); denominator s_i is accumulated with vector adds of the
    exp'ed tiles plus a final ones-vector matmul partition-reduce; the
    normalization 1/s_i is applied to the [C, N] output instead of to the
    [N, N] attention matrix (flash-attention style deferred normalization).
  - gamma is folded into Wv/bv on the host; residual "+ x" applied on-chip.

PSUM layout: all wide psum tiles are [128, 1024] fp32 = 2 banks; the two
logical halves live at element offsets 0 and 512 (bank-aligned) because a
single matmul output must not cross a 2KiB psum bank boundary.
"""

import numpy as np
import ml_dtypes

import bass_rust as _bass_rust

import concourse.bass as bass
import concourse.mybir as mybir
import concourse.tile as tile
from concourse.bass_utils import run_bass_kernel_spmd

BF16 = mybir.dt.bfloat16
F32 = mybir.dt.float32
NP_BF16 = ml_dtypes.bfloat16
AF = mybir.ActivationFunctionType

B, C, H, W = 8, 256, 56, 56
CI = 64
N = H * W            # 3136
NCORES = 8
PFD = 448            # projection chunk (Q/K): 7 * 448 = 3136
OFF2 = 512           # second-half offset inside [128, 1024] psum tiles
I_CHUNKS = [512] * 6 + [64]     # query chunks: 6*512 + 64 = 3136
NJ = 25              # j-chunks: 24 x 128 + 1 x 64
NPAIR = 12           # full pairs of 128-wide j-chunks


def build_kernel(n_repeat: int = 1) -> bass.Bass:
    nc = bass.Bass()

    xb_d = nc.declare_dram_parameter("xb", [C, N], BF16, isOutput=False)
    xf_d = nc.declare_dram_parameter("xf", [C, N], F32, isOutput=False)
    wq_d = nc.declare_dram_parameter("wqT", [C, CI], BF16, isOutput=False)
    wk_d = nc.declare_dram_parameter("wkT", [C, CI], BF16, isOutput=False)
    wv_d = nc.declare_dram_parameter("wvT", [C, C], BF16, isOutput=False)
    bq_d = nc.declare_dram_parameter("bq", [CI, 1], F32, isOutput=False)
    bk_d = nc.declare_dram_parameter("bk", [CI, 1], F32, isOutput=False)
    bv_d = nc.declare_dram_parameter("bv", [1, C], BF16, isOutput=False)
    out_d = nc.declare_dram_parameter("out", [C, N], F32, isOutput=True)

    xb_r = xb_d[:].rearrange("(o p) n -> p o n", p=128)    # [128, 2, N] bf16
    xf_r = xf_d[:].rearrange("(o p) n -> p o n", p=128)    # [128, 2, N] f32
    out_r = out_d[:].rearrange("(o p) n -> p o n", p=128)  # [128, 2, N] f32

    with tile.TileContext(nc) as tc:
        with (
            tc.tile_pool(name="const", bufs=1) as cpool,
            tc.tile_pool(name="data", bufs=1) as dpool,
            tc.tile_pool(name="att", bufs=3) as apool,
            tc.tile_pool(name="accp", bufs=2) as accpool,
            tc.tile_pool(name="outp", bufs=2) as opool,
            tc.tile_pool(name="misc", bufs=2) as mpool,
            tc.tile_pool(name="ps_a", bufs=2, space="PSUM") as ps_a,
            tc.tile_pool(name="ps_o", bufs=2, space="PSUM") as ps_o,
        ):
            # ---- constants / weights ----
            wq_sb = cpool.tile([128, 2, CI], BF16, name="wq_sb")
            nc.sync.dma_start(wq_sb[:], wq_d[:].rearrange("(o p) m -> p o m", p=128))
            wk_sb = cpool.tile([128, 2, CI], BF16, name="wk_sb")
            nc.sync.dma_start(wk_sb[:], wk_d[:].rearrange("(o p) m -> p o m", p=128))
            wv_sb = cpool.tile([128, 2, C], BF16, name="wv_sb")
            nc.sync.dma_start(wv_sb[:], wv_d[:].rearrange("(o p) m -> p o m", p=128))
            bq_sb = cpool.tile([CI, 1], F32, name="bq_sb")
            nc.sync.dma_start(bq_sb[:], bq_d[:])
            bk_sb = cpool.tile([CI, 1], F32, name="bk_sb")
            nc.sync.dma_start(bk_sb[:], bk_d[:])
            bv_sb = cpool.tile([1, C], BF16, name="bv_sb")
            nc.sync.dma_start(bv_sb[:], bv_d[:])
            ones_col = cpool.tile([128, 1], BF16, name="ones_col")
            nc.vector.memset(ones_col[:], 1.0)
            ones_rb = cpool.tile([1, 128], BF16, name="ones_rb")
            nc.vector.memset(ones_rb[:], 1.0)
            ones_rf = cpool.tile([1, 128], F32, name="ones_rf")
            nc.vector.memset(ones_rf[:], 1.0)

            # ---- x in SBUF ----
            xb_sb = dpool.tile([128, 2, N], BF16, name="xb_sb")
            nc.sync.dma_start(xb_sb[:], xb_r)
            xf_sb = dpool.tile([128, 2, N], F32, name="xf_sb")
            nc.sync.dma_start(xf_sb[:], xf_r)

            q_sb = dpool.tile([CI, N], BF16, name="q_sb")
            k_sb = dpool.tile([CI, N], BF16, name="k_sb")
            vt_sb = dpool.tile([128, NJ, C], BF16, name="vt_sb")

            for _rep in range(n_repeat):
                # ---- projections: Q and K, [CI, N] bf16 ----
                for t in range(N // PFD):
                    sl = slice(t * PFD, (t + 1) * PFD)
                    pq = ps_a.tile([128, 1024], F32, tag="ps_a")
                    nc.tensor.matmul(pq[:CI, 0:PFD], lhsT=wq_sb[:, 0, :],
                                     rhs=xb_sb[:, 0, sl], start=True, stop=False)
                    nc.tensor.matmul(pq[:CI, 0:PFD], lhsT=wq_sb[:, 1, :],
                                     rhs=xb_sb[:, 1, sl], start=False, stop=True)
                    nc.tensor.matmul(pq[:CI, OFF2:OFF2 + PFD], lhsT=wk_sb[:, 0, :],
                                     rhs=xb_sb[:, 0, sl], start=True, stop=False)
                    nc.tensor.matmul(pq[:CI, OFF2:OFF2 + PFD], lhsT=wk_sb[:, 1, :],
                                     rhs=xb_sb[:, 1, sl], start=False, stop=True)
                    nc.scalar.activation(q_sb[:, sl], pq[:CI, 0:PFD],
                                         AF.Identity, bias=bq_sb[:])
                    nc.scalar.activation(k_sb[:, sl], pq[:CI, OFF2:OFF2 + PFD],
                                         AF.Identity, bias=bk_sb[:])

                # ---- V^T tiles: vt_sb[p, jt, c] = gamma*V[c, jt*128+p] ----
                for jt in range(NJ):
                    jsz = 128 if jt < NJ - 1 else 64
                    j0 = jt * 128
                    pv = ps_a.tile([128, 1024], F32, tag="ps_a")
                    pvt = pv[:jsz, 0:C]
                    nc.tensor.matmul(pvt, lhsT=xb_sb[:, 0, j0:j0 + jsz],
                                     rhs=wv_sb[:, 0, :], start=True, stop=False)
                    nc.tensor.matmul(pvt, lhsT=xb_sb[:, 1, j0:j0 + jsz],
                                     rhs=wv_sb[:, 1, :], start=False, stop=False)
                    nc.tensor.matmul(pvt, lhsT=ones_rb[:, :jsz],
                                     rhs=bv_sb[:], start=False, stop=True)
                    nc.vector.tensor_copy(vt_sb[:jsz, jt, :], pvt)

                # ---- attention, one i-chunk at a time ----
                i0 = 0
                for fd in I_CHUNKS:
                    isl = slice(i0, i0 + fd)
                    full = fd == 512
                    po = ps_o.tile([128, 1024], F32, tag="ps_o")
                    acc = accpool.tile([128, 1024], BF16, tag="acc")
                    nc.vector.memset(acc[:], 0.0)

                    for jp in range(NPAIR + 1):
                        ps = ps_a.tile([128, 1024], F32, tag="ps_a")
                        att = apool.tile([128, 1024], BF16, tag="att")
                        if jp < NPAIR:
                            j0 = (2 * jp) * 128
                            j1 = (2 * jp + 1) * 128
                            nc.tensor.matmul(ps[:, 0:fd], lhsT=k_sb[:, j0:j0 + 128],
                                             rhs=q_sb[:, isl], start=True, stop=True)
                            nc.tensor.matmul(ps[:, OFF2:OFF2 + fd],
                                             lhsT=k_sb[:, j1:j1 + 128],
                                             rhs=q_sb[:, isl], start=True, stop=True)
                            if full:
                                nc.scalar.activation(att[:], ps[:], AF.Exp)
                                nc.vector.tensor_add(acc[:], acc[:], att[:])
                            else:
                                nc.scalar.activation(att[:, 0:fd], ps[:, 0:fd], AF.Exp)
                                nc.scalar.activation(att[:, OFF2:OFF2 + fd],
                                                     ps[:, OFF2:OFF2 + fd], AF.Exp)
                                nc.vector.tensor_add(acc[:, 0:fd], acc[:, 0:fd],
                                                     att[:, 0:fd])
                                nc.vector.tensor_add(acc[:, OFF2:OFF2 + fd],
                                                     acc[:, OFF2:OFF2 + fd],
                                                     att[:, OFF2:OFF2 + fd])
                            for cc in range(2):
                                osl = slice(cc * OFF2, cc * OFF2 + fd)
                                csl = slice(cc * 128, (cc + 1) * 128)
                                nc.tensor.matmul(po[:, osl],
                                                 lhsT=vt_sb[:, 2 * jp, csl],
                                                 rhs=att[:, 0:fd],
                                                 start=(jp == 0), stop=False)
                                nc.tensor.matmul(po[:, osl],
                                                 lhsT=vt_sb[:, 2 * jp + 1, csl],
                                                 rhs=att[:, OFF2:OFF2 + fd],
                                                 start=False, stop=False)
                        else:
                            # last j-chunk: 64 keys
                            j0 = NPAIR * 2 * 128  # 3072
                            jsz = 64
                            nc.tensor.matmul(ps[:jsz, 0:fd], lhsT=k_sb[:, j0:j0 + jsz],
                                             rhs=q_sb[:, isl], start=True, stop=True)
                            nc.scalar.activation(att[:jsz, 0:fd], ps[:jsz, 0:fd],
                                                 AF.Exp)
                            nc.vector.tensor_add(acc[:jsz, 0:fd], acc[:jsz, 0:fd],
                                                 att[:jsz, 0:fd])
                            for cc in range(2):
                                osl = slice(cc * OFF2, cc * OFF2 + fd)
                                csl = slice(cc * 128, (cc + 1) * 128)
                                nc.tensor.matmul(po[:, osl],
                                                 lhsT=vt_sb[:jsz, NJ - 1, csl],
                                                 rhs=att[:jsz, 0:fd],
                                                 start=False, stop=True)

                    # softmax denominator s_i = sum_j exp(score[i, j])
                    ps1 = ps_a.tile([128, 1024], F32, tag="ps_a")
                    s1 = ps1[:1, 0:fd]
                    nc.tensor.matmul(s1, lhsT=ones_col[:], rhs=acc[:, 0:fd],
                                     start=True, stop=False)
                    nc.tensor.matmul(s1, lhsT=ones_col[:], rhs=acc[:, OFF2:OFF2 + fd],
                                     start=False, stop=True)
                    inv_sb = mpool.tile([1, OFF2], F32, tag="inv")
                    nc.vector.reciprocal(inv_sb[:, :fd], s1)
                    # broadcast 1/s to 128 partitions via K=1 matmul
                    pb = ps1[:, OFF2:OFF2 + fd]
                    nc.tensor.matmul(pb, lhsT=ones_rf[:], rhs=inv_sb[:, :fd],
                                     start=True, stop=True)
                    invbc = mpool.tile([128, OFF2], F32, tag="invbc")
                    nc.vector.tensor_copy(invbc[:, :fd], pb)

                    out_sb = opool.tile([128, 2, OFF2], F32, tag="out")
                    for cc in range(2):
                        nc.vector.tensor_mul(out_sb[:, cc, :fd],
                                             po[:, cc * OFF2:cc * OFF2 + fd],
                                             invbc[:, :fd])
                        nc.gpsimd.tensor_add(out_sb[:, cc, :fd], out_sb[:, cc, :fd],
                                             xf_sb[:, cc, isl])
                    nc.sync.dma_start(out_r[:, :, isl], out_sb[:, :, :fd])
                    i0 += fd

    # TRN2 allows at most one semaphore wait per instruction; Tile can emit
    # more. Split them (EventSemaphore chains) like Bacc.compile() does.
    _bass_rust.move_matmul_waits_to_ldweights(nc.m)
    _bass_rust.generate_event_semaphores(nc)
    return nc


_CACHED = {}


def _get_kernel(n_repeat: int = 1) -> bass.Bass:
    if n_repeat not in _CACHED:
        _CACHED[n_repeat] = build_kernel(n_repeat)
    return _CACHED[n_repeat]


def make_in_maps(x, Wq, bq, Wk, bk, Wv, bv, gamma):
    x = np.asarray(x, dtype=np.float32)
    Wq = np.asarray(Wq, dtype=np.float32)
    bq = np.asarray(bq, dtype=np.float32)
    Wk = np.asarray(Wk, dtype=np.float32)
    bk = np.asarray(bk, dtype=np.float32)
    Wv = np.asarray(Wv, dtype=np.float32)
    bv = np.asarray(bv, dtype=np.float32)
    g = float(np.asarray(gamma, dtype=np.float32).reshape(-1)[0])

    wqT = np.ascontiguousarray(Wq.T).astype(NP_BF16)            # [C, CI]
    wkT = np.ascontiguousarray(Wk.T).astype(NP_BF16)            # [C, CI]
    wvT = np.ascontiguousarray((g * Wv).T).astype(NP_BF16)      # [C, C]
    bq2 = np.ascontiguousarray(bq.reshape(CI, 1))               # [CI, 1] f32
    bk2 = np.ascontiguousarray(bk.reshape(CI, 1))
    bv2 = np.ascontiguousarray((g * bv).reshape(1, C)).astype(NP_BF16)

    xf = np.ascontiguousarray(x.reshape(B, C, N))
    xbf = xf.astype(NP_BF16)

    in_maps = []
    for b in range(B):
        in_maps.append({
            "xb": xbf[b],
            "xf": xf[b],
            "wqT": wqT,
            "wkT": wkT,
            "wvT": wvT,
            "bq": bq2,
            "bk": bk2,
            "bv": bv2,
        })
    return in_maps


def kernel(x, Wq, bq, Wk, bk, Wv, bv, gamma):
    in_maps = make_in_maps(x, Wq, bq, Wk, bk, Wv, bv, gamma)
    nc = _get_kernel(1)
    res = run_bass_kernel_spmd(nc, in_maps, core_ids=list(range(NCORES)))
    out = np.stack([res.results[b]["out"] for b in range(B)], axis=0)
    return out.reshape(B, C, H, W).astype(np.float32)
